# revision 1
# baseline (speedup 1.0000x reference)
"""GRU layer (flax GRUCell math) on 8 Trainium2 NeuronCores.

Data-parallel: batch 64 sharded 8-way (8 rows/core); weights replicated;
the T=4096 recurrence runs locally per core.

Per-core layout: hidden state kept as h^T [H=128 partitions, B_loc=8 free].
Per chunk of C=64 steps:
  - x chunk DMA'd in [t*b, d] blocks, PE-transposed to x^T [d, t*b]
  - gate pre-activations for the whole chunk accumulate in PSUM:
      bias broadcast (K=1 matmul) + x-side GEMM (N=512) + per-step h-side
      matmul (N=8), so sigmoid/tanh inputs come straight from PSUM.
  - n-gate: v = (h Whn + b_hn) * r on DVE, w = v + gn on DVE, n = tanh(w)
    on ScalarE, h' = n + z*(h-n) split across GpSimd/DVE.
  - h' written into a [H, C*B] staging tile (also the next step's matmul
    rhs), PE-transposed at chunk end and DMA'd to DRAM.
"""

import sys

sys.path.insert(0, "/opt/trn_rl_repo")

import numpy as np

import concourse.bacc as bacc
import concourse.tile as tile
from concourse import mybir
from concourse.masks import make_identity
from concourse.bass_utils import run_bass_kernel_spmd

F32 = mybir.dt.float32
AF = mybir.ActivationFunctionType

B, T, D, H = 64, 4096, 128, 128
NCORES = 8
BL = B // NCORES  # 8 batch rows per core


def build_gru_nc(BL=BL, T=T, C=64):
    """Build the single-core GRU program (SPMD-replicated across cores)."""
    assert T % C == 0 and C % 16 == 0
    # r|z gate regions must land in distinct PSUM banks: a start=True matmul
    # clears has_written for its whole bank, so each bank gets exactly one.
    assert (C * BL * 4) % 2048 == 0, "chunk region must be a whole PSUM bank"
    NCH = T // C
    BT = C * BL  # columns per chunk in [H, t*b] staging layout
    NBLK = BT // 128  # 128-column blocks per chunk (transposes)
    TBLK = 128 // BL  # time steps per 128-column block

    nc = bacc.Bacc("TRN2", target_bir_lowering=False, debug=False)

    x_d = nc.dram_tensor("x", [BL, T, D], F32, kind="ExternalInput").ap()
    wi_d = nc.dram_tensor("wi", [D, 3 * H], F32, kind="ExternalInput").ap()
    wh_d = nc.dram_tensor("wh", [H, 3 * H], F32, kind="ExternalInput").ap()
    # b_row = [b_ir | b_iz | b_hn] as a row vector for K=1 broadcast matmuls
    brow_d = nc.dram_tensor("b_row", [1, 3 * H], F32, kind="ExternalInput").ap()
    bin_d = nc.dram_tensor("b_in", [H, 1], F32, kind="ExternalInput").ap()
    y_d = nc.dram_tensor("y", [BL, T, H], F32, kind="ExternalOutput").ap()

    x_tbd = x_d.rearrange("b t d -> t b d")
    y_tbh = y_d.rearrange("b t h -> t b h")

    with tile.TileContext(nc) as tc:
        with (
            tc.tile_pool(name="const", bufs=1) as const_p,
            tc.tile_pool(name="xraw", bufs=2 * NBLK) as xraw_p,
            tc.tile_pool(name="xt", bufs=2) as xt_p,
            tc.tile_pool(name="gn", bufs=2) as gn_p,
            tc.tile_pool(name="hs", bufs=2) as hs_p,
            tc.tile_pool(name="outt", bufs=2 * NBLK) as outt_p,
            tc.tile_pool(name="small", bufs=8) as small_p,
            tc.tile_pool(name="prz", bufs=2, space="PSUM") as prz_p,
            tc.tile_pool(name="pnh", bufs=2, space="PSUM") as pnh_p,
            tc.tile_pool(name="pscr", bufs=2, space="PSUM") as pscr_p,
        ):
            wi = const_p.tile([D, 3 * H], F32)
            nc.sync.dma_start(wi[:], wi_d)
            wh = const_p.tile([H, 3 * H], F32)
            nc.sync.dma_start(wh[:], wh_d)
            brow = const_p.tile([1, 3 * H], F32)
            nc.sync.dma_start(brow[:], brow_d)
            bin_ = const_p.tile([H, 1], F32)
            nc.sync.dma_start(bin_[:], bin_d)
            ones = const_p.tile([1, BT], F32)
            nc.vector.memset(ones[:], 1.0)
            ident = const_p.tile([128, 128], F32)
            make_identity(nc, ident[:])
            hinit = const_p.tile([H, BL], F32)
            nc.vector.memset(hinit[:], 0.0)

            prev_stage = None
            for c in range(NCH):
                t0 = c * C

                # ---- prep: x chunk in, transpose, gate GEMMs into PSUM ----
                xt = xt_p.tile([D, BT], F32)
                for k in range(NBLK):
                    xr = xraw_p.tile([128, 128], F32, tag="xraw")
                    nc.sync.dma_start(
                        xr[:],
                        x_tbd[t0 + TBLK * k : t0 + TBLK * (k + 1)],
                    )
                    ps = pscr_p.tile([128, BT], F32, tag="scr")
                    nc.tensor.transpose(ps[:, 0:128], xr[:], ident[:])
                    nc.vector.tensor_copy(xt[:, 128 * k : 128 * (k + 1)], ps[:, 0:128])

                prz = prz_p.tile([128, 2 * BT], F32)
                pnh = pnh_p.tile([128, BT], F32)
                nc.tensor.matmul(prz[:, 0:BT], brow[:, 0:H], ones[:], start=True, stop=False)
                nc.tensor.matmul(prz[:, BT : 2 * BT], brow[:, H : 2 * H], ones[:], start=True, stop=False)
                nc.tensor.matmul(pnh[:], brow[:, 2 * H : 3 * H], ones[:], start=True, stop=False)
                nc.tensor.matmul(prz[:, 0:BT], wi[:, 0:H], xt[:], start=False, stop=False)
                nc.tensor.matmul(prz[:, BT : 2 * BT], wi[:, H : 2 * H], xt[:], start=False, stop=False)
                pgn = pscr_p.tile([128, BT], F32, tag="scr")
                nc.tensor.matmul(pgn[:], wi[:, 2 * H : 3 * H], xt[:], start=True, stop=True)
                gn = gn_p.tile([128, BT], F32)
                nc.scalar.activation(gn[:], pgn[:], AF.Identity, bias=bin_[:])

                # ---- scan ----
                stage = hs_p.tile([H, BT], F32)
                prz3 = prz[:].rearrange("p (g c) -> p g c", g=2)
                for tl in range(C):
                    if c == 0 and tl == 0:
                        h_ap = hinit[:]
                    elif tl == 0:
                        h_ap = prev_stage[:, (C - 1) * BL : C * BL]
                    else:
                        h_ap = stage[:, (tl - 1) * BL : tl * BL]
                    cs = slice(tl * BL, (tl + 1) * BL)
                    zs = slice(BT + tl * BL, BT + (tl + 1) * BL)
                    nc.tensor.matmul(prz[:, cs], wh[:, 0:H], h_ap, start=False, stop=True)
                    nc.tensor.matmul(prz[:, zs], wh[:, H : 2 * H], h_ap, start=False, stop=True)
                    nc.tensor.matmul(pnh[:, cs], wh[:, 2 * H : 3 * H], h_ap, start=False, stop=True)

                    rz = small_p.tile([H, 2 * BL], F32, tag="rz")
                    nc.scalar.activation(
                        rz[:].rearrange("p (g c) -> p g c", g=2),
                        prz3[:, :, cs],
                        AF.Sigmoid,
                    )
                    v = small_p.tile([H, BL], F32, tag="v")
                    nc.vector.tensor_mul(v[:], pnh[:, cs], rz[:, 0:BL])
                    w = small_p.tile([H, BL], F32, tag="w")
                    nc.vector.tensor_add(w[:], v[:], gn[:, cs])
                    n = small_p.tile([H, BL], F32, tag="n")
                    nc.scalar.activation(n[:], w[:], AF.Tanh)
                    d = small_p.tile([H, BL], F32, tag="d")
                    nc.vector.tensor_sub(d[:], h_ap, n[:])
                    e = small_p.tile([H, BL], F32, tag="e")
                    nc.vector.tensor_mul(e[:], rz[:, BL : 2 * BL], d[:])
                    nc.vector.tensor_add(stage[:, cs], n[:], e[:])

                # ---- epilogue: transpose staging back and DMA out ----
                for k in range(NBLK):
                    ps = pscr_p.tile([128, BT], F32, tag="scr")
                    nc.tensor.transpose(ps[:, 0:128], stage[:, 128 * k : 128 * (k + 1)], ident[:])
                    ot = outt_p.tile([128, 128], F32, tag="outt")
                    nc.vector.tensor_copy(ot[:], ps[:, 0:128])
                    nc.sync.dma_start(
                        y_tbh[t0 + TBLK * k : t0 + TBLK * (k + 1)],
                        ot[:],
                    )
                prev_stage = stage

    nc.compile()
    return nc


_NC_CACHE = {}


def _get_nc(BL_, T_, C_):
    key = (BL_, T_, C_)
    if key not in _NC_CACHE:
        _NC_CACHE[key] = build_gru_nc(BL_, T_, C_)
    return _NC_CACHE[key]


def run_gru(x, Wir, Wiz, Win, Whr, Whz, Whn, b_ir, b_iz, b_in, b_hn, C=64, trace=False):
    """x: [B, T, D] float32 (B divisible by NCORES). Returns [B, T, H], plus results obj."""
    x = np.ascontiguousarray(np.asarray(x, dtype=np.float32))
    Bx, Tx, Dx = x.shape
    bl = Bx // NCORES
    wi = np.ascontiguousarray(np.concatenate([Wir, Wiz, Win], axis=1).astype(np.float32))
    wh = np.ascontiguousarray(np.concatenate([Whr, Whz, Whn], axis=1).astype(np.float32))
    brow = np.ascontiguousarray(
        np.concatenate([b_ir, b_iz, b_hn])[None, :].astype(np.float32)
    )
    bin_ = np.ascontiguousarray(np.asarray(b_in, dtype=np.float32)[:, None])

    nc = _get_nc(bl, Tx, C)
    in_maps = [
        {
            "x": x[i * bl : (i + 1) * bl],
            "wi": wi,
            "wh": wh,
            "b_row": brow,
            "b_in": bin_,
        }
        for i in range(NCORES)
    ]
    res = run_bass_kernel_spmd(nc, in_maps, list(range(NCORES)), trace=trace)
    y = np.concatenate([res.results[i]["y"] for i in range(NCORES)], axis=0)
    return y, res


def kernel(**inputs) -> np.ndarray:
    inputs = {k: np.asarray(v) for k, v in inputs.items()}
    y, _ = run_gru(**inputs)
    return y.astype(np.float32)


if __name__ == "__main__":
    # smoke test with tiny T against a local numpy GRU reference
    rng = np.random.default_rng(0)
    Ts = 128
    s_i, s_h = 1.0 / np.sqrt(D), 1.0 / np.sqrt(H)
    inp = {
        "x": rng.standard_normal((B, Ts, D), dtype=np.float32),
        "Wir": rng.uniform(-s_i, s_i, (D, H)).astype(np.float32),
        "Wiz": rng.uniform(-s_i, s_i, (D, H)).astype(np.float32),
        "Win": rng.uniform(-s_i, s_i, (D, H)).astype(np.float32),
        "Whr": rng.uniform(-s_h, s_h, (H, H)).astype(np.float32),
        "Whz": rng.uniform(-s_h, s_h, (H, H)).astype(np.float32),
        "Whn": rng.uniform(-s_h, s_h, (H, H)).astype(np.float32),
        "b_ir": rng.uniform(-s_i, s_i, (H,)).astype(np.float32),
        "b_iz": rng.uniform(-s_i, s_i, (H,)).astype(np.float32),
        "b_in": rng.uniform(-s_i, s_i, (H,)).astype(np.float32),
        "b_hn": rng.uniform(-s_h, s_h, (H,)).astype(np.float32),
    }

    def np_gru(x, Wir, Wiz, Win, Whr, Whz, Whn, b_ir, b_iz, b_in, b_hn):
        Bx, Tx, _ = x.shape
        h = np.zeros((Bx, H), np.float32)
        gi_r = x @ Wir + b_ir
        gi_z = x @ Wiz + b_iz
        gi_n = x @ Win + b_in
        out = np.zeros((Bx, Tx, H), np.float32)
        for t in range(Tx):
            r = 1 / (1 + np.exp(-(gi_r[:, t] + h @ Whr)))
            z = 1 / (1 + np.exp(-(gi_z[:, t] + h @ Whz)))
            n = np.tanh(gi_n[:, t] + r * (h @ Whn + b_hn))
            h = (1 - z) * n + z * h
            out[:, t] = h
        return out

    expected = np_gru(**inp)
    y, _ = run_gru(**inp, C=64)
    err = np.abs(y - expected).max() / (np.abs(expected).max() + 1e-30)
    print("max abs err (rel to absmax):", err)
    assert err < 2e-3, err
    print("SMOKE TEST PASSED")



# revision 2
# speedup vs baseline: 1.0058x; 1.0058x over previous
"""GRU layer (flax GRUCell math) on 8 Trainium2 NeuronCores.

Data-parallel: batch 64 sharded 8-way (8 rows/core); weights replicated;
the T=4096 recurrence runs locally per core.

Per-core layout: hidden state h^T [H=128 partitions, B_loc=8 free], stored
bf16. All matmul operands (weights, x projections, h state) are bf16 --
PE weight-load and streaming run at 1 cycle/column instead of fp32's 2-4,
which is the dominant critical-path cost at low PE p-state. PSUM
accumulation and all gate/elementwise math stay fp32 (rel err ~6e-3 from
bf16 state quantization, budget 2e-2).

Per chunk of C=64 steps: x DMA'd in [t*b, d] blocks, cast to bf16,
PE-transposed via one shared 1-bank PSUM scratch; gate pre-activations
accumulate in PSUM (bias K=1 matmul + x GEMM + per-step h matmul), so
sigmoid reads straight from PSUM.

Per step: 3 bf16 matmuls -> sigmoid(r|z) [ACT] -> v=phn*r, w=v+gn [DVE]
-> tanh [ACT] -> a=(z-1)*n, h'=b-a [DVE], with b=z*h and zm1=z-1 computed
during the tanh window off the critical path.
"""

import sys

sys.path.insert(0, "/opt/trn_rl_repo")

import numpy as np

import concourse.bacc as bacc
import concourse.tile as tile
from concourse import mybir
from concourse.masks import make_identity
from concourse.bass_utils import run_bass_kernel_spmd

F32 = mybir.dt.float32
BF16 = mybir.dt.bfloat16
AF = mybir.ActivationFunctionType

B, T, D, H = 64, 4096, 128, 128
NCORES = 8
BL = B // NCORES  # 8 batch rows per core


def build_gru_nc(BL=BL, T=T, C=64):
    """Build the single-core GRU program (SPMD-replicated across cores)."""
    assert T % C == 0 and C % 16 == 0
    # r|z gate regions must land in distinct PSUM banks: a start=True matmul
    # clears has_written for its whole bank, so each bank gets exactly one.
    assert (C * BL * 4) % 2048 == 0, "chunk region must be a whole PSUM bank"
    NCH = T // C
    BT = C * BL  # columns per chunk in [H, t*b] staging layout
    NBLK = BT // 128  # 128-column blocks per chunk (transposes)
    TBLK = 128 // BL  # time steps per 128-column block

    nc = bacc.Bacc("TRN2", target_bir_lowering=False, debug=False)

    x_d = nc.dram_tensor("x", [BL, T, D], F32, kind="ExternalInput").ap()
    wi_d = nc.dram_tensor("wi", [D, 3 * H], F32, kind="ExternalInput").ap()
    wh_d = nc.dram_tensor("wh", [H, 3 * H], F32, kind="ExternalInput").ap()
    # b_row = [b_ir | b_iz | b_hn] as a row vector for K=1 broadcast matmuls
    brow_d = nc.dram_tensor("b_row", [1, 3 * H], F32, kind="ExternalInput").ap()
    bin_d = nc.dram_tensor("b_in", [H, 1], F32, kind="ExternalInput").ap()
    y_d = nc.dram_tensor("y", [BL, T, H], F32, kind="ExternalOutput").ap()

    x_tbd = x_d.rearrange("b t d -> t b d")
    y_tbh = y_d.rearrange("b t h -> t b h")

    with tile.TileContext(nc) as tc:
        with (
            tc.tile_pool(name="const", bufs=1) as const_p,
            tc.tile_pool(name="xraw", bufs=2 * NBLK) as xraw_p,
            tc.tile_pool(name="xt", bufs=2) as xt_p,
            tc.tile_pool(name="gn", bufs=2) as gn_p,
            tc.tile_pool(name="hs", bufs=2) as hs_p,
            tc.tile_pool(name="outt", bufs=2 * NBLK) as outt_p,
            tc.tile_pool(name="small", bufs=8) as small_p,
            tc.tile_pool(name="prz", bufs=2, space="PSUM") as prz_p,
            tc.tile_pool(name="pnh", bufs=2, space="PSUM") as pnh_p,
            tc.tile_pool(name="pscr", bufs=1, space="PSUM") as pscr_p,
        ):
            wi32 = const_p.tile([D, 3 * H], F32)
            nc.sync.dma_start(wi32[:], wi_d)
            wi = const_p.tile([D, 3 * H], BF16)
            nc.vector.tensor_copy(wi[:], wi32[:])
            wh32 = const_p.tile([H, 3 * H], F32)
            nc.sync.dma_start(wh32[:], wh_d)
            wh = const_p.tile([H, 3 * H], BF16)
            nc.vector.tensor_copy(wh[:], wh32[:])
            brow32 = const_p.tile([1, 3 * H], F32)
            nc.sync.dma_start(brow32[:], brow_d)
            brow = const_p.tile([1, 3 * H], BF16)
            nc.vector.tensor_copy(brow[:], brow32[:])
            bin_ = const_p.tile([H, 1], F32)
            nc.sync.dma_start(bin_[:], bin_d)
            ones = const_p.tile([1, BT], BF16)
            nc.vector.memset(ones[:], 1.0)
            ident = const_p.tile([128, 128], F32)
            make_identity(nc, ident[:])
            identb = const_p.tile([128, 128], BF16)
            nc.vector.tensor_copy(identb[:], ident[:])
            hinit = const_p.tile([H, BL], BF16)
            nc.vector.memset(hinit[:], 0.0)

            prev_stage = None
            for c in range(NCH):
                t0 = c * C

                # ---- prep: x chunk in, transpose, gate GEMMs into PSUM ----
                xt = xt_p.tile([D, BT], BF16)
                for k in range(NBLK):
                    xr = xraw_p.tile([128, 128], F32, tag="xraw")
                    nc.sync.dma_start(
                        xr[:],
                        x_tbd[t0 + TBLK * k : t0 + TBLK * (k + 1)],
                    )
                    xrb = xraw_p.tile([128, 128], BF16, tag="xrawb")
                    nc.vector.tensor_copy(xrb[:], xr[:])
                    psb = pscr_p.tile([128, 128], BF16, tag="scrb", bufs=1)
                    nc.tensor.transpose(psb[:], xrb[:], identb[:])
                    nc.vector.tensor_copy(xt[:, 128 * k : 128 * (k + 1)], psb[:])

                prz = prz_p.tile([128, 2 * BT], F32)
                pnh = pnh_p.tile([128, BT], F32)
                nc.tensor.matmul(prz[:, 0:BT], brow[:, 0:H], ones[:], start=True, stop=False)
                nc.tensor.matmul(prz[:, BT : 2 * BT], brow[:, H : 2 * H], ones[:], start=True, stop=False)
                nc.tensor.matmul(pnh[:], brow[:, 2 * H : 3 * H], ones[:], start=True, stop=False)
                nc.tensor.matmul(prz[:, 0:BT], wi[:, 0:H], xt[:], start=False, stop=False)
                nc.tensor.matmul(prz[:, BT : 2 * BT], wi[:, H : 2 * H], xt[:], start=False, stop=False)
                pgn = pscr_p.tile([128, BT], F32, tag="scr", bufs=1)
                nc.tensor.matmul(pgn[:], wi[:, 2 * H : 3 * H], xt[:], start=True, stop=True)
                gn = gn_p.tile([128, BT], F32)
                nc.scalar.activation(gn[:], pgn[:], AF.Identity, bias=bin_[:])

                # ---- scan ----
                stage = hs_p.tile([H, BT], BF16)
                prz3 = prz[:].rearrange("p (g c) -> p g c", g=2)
                for tl in range(C):
                    if c == 0 and tl == 0:
                        h_ap = hinit[:]
                    elif tl == 0:
                        h_ap = prev_stage[:, (C - 1) * BL : C * BL]
                    else:
                        h_ap = stage[:, (tl - 1) * BL : tl * BL]
                    cs = slice(tl * BL, (tl + 1) * BL)
                    zs = slice(BT + tl * BL, BT + (tl + 1) * BL)
                    nc.tensor.matmul(prz[:, cs], wh[:, 0:H], h_ap, start=False, stop=True)
                    nc.tensor.matmul(prz[:, zs], wh[:, H : 2 * H], h_ap, start=False, stop=True)
                    nc.tensor.matmul(pnh[:, cs], wh[:, 2 * H : 3 * H], h_ap, start=False, stop=True)

                    rz = small_p.tile([H, 2 * BL], F32, tag="rz")
                    nc.scalar.activation(
                        rz[:].rearrange("p (g c) -> p g c", g=2),
                        prz3[:, :, cs],
                        AF.Sigmoid,
                    )
                    v = small_p.tile([H, BL], F32, tag="v")
                    nc.vector.tensor_mul(v[:], pnh[:, cs], rz[:, 0:BL])
                    w = small_p.tile([H, BL], F32, tag="w")
                    nc.vector.tensor_add(w[:], v[:], gn[:, cs])
                    # off-path during tanh window: b = z*h, zm1 = z-1
                    b_t = small_p.tile([H, BL], F32, tag="b")
                    nc.vector.tensor_mul(b_t[:], rz[:, BL : 2 * BL], h_ap)
                    zm1 = small_p.tile([H, BL], F32, tag="zm1")
                    nc.vector.tensor_scalar_add(zm1[:], rz[:, BL : 2 * BL], -1.0)
                    n = small_p.tile([H, BL], F32, tag="n")
                    nc.scalar.activation(n[:], w[:], AF.Tanh)
                    a_t = small_p.tile([H, BL], F32, tag="a")
                    nc.vector.tensor_mul(a_t[:], zm1[:], n[:])
                    nc.vector.tensor_sub(stage[:, cs], b_t[:], a_t[:])

                # ---- epilogue: transpose staging back and DMA out ----
                for k in range(NBLK):
                    psb = pscr_p.tile([128, 128], BF16, tag="scrb", bufs=1)
                    nc.tensor.transpose(psb[:], stage[:, 128 * k : 128 * (k + 1)], identb[:])
                    ot = outt_p.tile([128, 128], F32, tag="outt")
                    nc.vector.tensor_copy(ot[:], psb[:])
                    nc.sync.dma_start(
                        y_tbh[t0 + TBLK * k : t0 + TBLK * (k + 1)],
                        ot[:],
                    )
                prev_stage = stage

    nc.compile()
    return nc


_NC_CACHE = {}


def _get_nc(BL_, T_, C_):
    key = (BL_, T_, C_)
    if key not in _NC_CACHE:
        _NC_CACHE[key] = build_gru_nc(BL_, T_, C_)
    return _NC_CACHE[key]


def run_gru(x, Wir, Wiz, Win, Whr, Whz, Whn, b_ir, b_iz, b_in, b_hn, C=64, trace=False):
    """x: [B, T, D] float32 (B divisible by NCORES). Returns [B, T, H], plus results obj."""
    x = np.ascontiguousarray(np.asarray(x, dtype=np.float32))
    Bx, Tx, Dx = x.shape
    bl = Bx // NCORES
    wi = np.ascontiguousarray(np.concatenate([Wir, Wiz, Win], axis=1).astype(np.float32))
    wh = np.ascontiguousarray(np.concatenate([Whr, Whz, Whn], axis=1).astype(np.float32))
    brow = np.ascontiguousarray(
        np.concatenate([b_ir, b_iz, b_hn])[None, :].astype(np.float32)
    )
    bin_ = np.ascontiguousarray(np.asarray(b_in, dtype=np.float32)[:, None])

    nc = _get_nc(bl, Tx, C)
    in_maps = [
        {
            "x": x[i * bl : (i + 1) * bl],
            "wi": wi,
            "wh": wh,
            "b_row": brow,
            "b_in": bin_,
        }
        for i in range(NCORES)
    ]
    res = run_bass_kernel_spmd(nc, in_maps, list(range(NCORES)), trace=trace)
    y = np.concatenate([res.results[i]["y"] for i in range(NCORES)], axis=0)
    return y, res


def kernel(**inputs) -> np.ndarray:
    inputs = {k: np.asarray(v) for k, v in inputs.items()}
    y, _ = run_gru(**inputs)
    return y.astype(np.float32)


if __name__ == "__main__":
    # smoke test with tiny T against a local numpy GRU reference
    rng = np.random.default_rng(0)
    Ts = 128
    s_i, s_h = 1.0 / np.sqrt(D), 1.0 / np.sqrt(H)
    inp = {
        "x": rng.standard_normal((B, Ts, D), dtype=np.float32),
        "Wir": rng.uniform(-s_i, s_i, (D, H)).astype(np.float32),
        "Wiz": rng.uniform(-s_i, s_i, (D, H)).astype(np.float32),
        "Win": rng.uniform(-s_i, s_i, (D, H)).astype(np.float32),
        "Whr": rng.uniform(-s_h, s_h, (H, H)).astype(np.float32),
        "Whz": rng.uniform(-s_h, s_h, (H, H)).astype(np.float32),
        "Whn": rng.uniform(-s_h, s_h, (H, H)).astype(np.float32),
        "b_ir": rng.uniform(-s_i, s_i, (H,)).astype(np.float32),
        "b_iz": rng.uniform(-s_i, s_i, (H,)).astype(np.float32),
        "b_in": rng.uniform(-s_i, s_i, (H,)).astype(np.float32),
        "b_hn": rng.uniform(-s_h, s_h, (H,)).astype(np.float32),
    }

    def np_gru(x, Wir, Wiz, Win, Whr, Whz, Whn, b_ir, b_iz, b_in, b_hn):
        Bx, Tx, _ = x.shape
        h = np.zeros((Bx, H), np.float32)
        gi_r = x @ Wir + b_ir
        gi_z = x @ Wiz + b_iz
        gi_n = x @ Win + b_in
        out = np.zeros((Bx, Tx, H), np.float32)
        for t in range(Tx):
            r = 1 / (1 + np.exp(-(gi_r[:, t] + h @ Whr)))
            z = 1 / (1 + np.exp(-(gi_z[:, t] + h @ Whz)))
            n = np.tanh(gi_n[:, t] + r * (h @ Whn + b_hn))
            h = (1 - z) * n + z * h
            out[:, t] = h
        return out

    expected = np_gru(**inp)
    y, _ = run_gru(**inp, C=64)
    err = np.abs(y - expected).max() / (np.abs(expected).max() + 1e-30)
    print("max abs err (rel to absmax):", err)
    assert err < 1.5e-2, err
    print("SMOKE TEST PASSED")

